# revision 10
# baseline (speedup 1.0000x reference)
"""Trainium2 Bass kernel for nn_ByteEncoder (entropy-gated byte patching encoder).

Sharding (8 NeuronCores, pure data parallel, no collectives):
  1. Entropy stage: the 16384-byte sequence is split 2048 bytes/core; each core
     runs the entropy MLP (gelu(x@W1.T)@W2.T -> log_softmax -> normalized
     entropy) with bf16 matmuls / f32 PSUM accumulation.
  2. Host: entropy-gated segmentation scan (O(S) integer logic) + patch
     packing + gathers.
  3. Encoder stage: real patches are split 128/core (1024 per launch); each
     core runs the 2-layer post-norm transformer encoder on 2048 tokens
     (16 superblocks x 128 tokens, block-diagonal 16-token attention via
     multiplicative masking), then mean-pools per patch on the PE.
Activations/weights bf16, PSUM f32; layernorm stats, softmax sums and the
patch pooling matmul in f32.
"""

import numpy as np
import ml_dtypes

import concourse.bass as bass
import concourse.tile as tile
import concourse.mybir as mybir
from concourse import bacc
from concourse import bass_utils
from concourse.bass import ts, ds
from concourse.masks import make_identity

# ---------------- problem constants (hardcoded per contract) ----------------
N_CORES = 8
D = 512
V = 256
LMAX = 16
MINP = 4
THR = 0.5
H = 8
HD = 64
F = 2048
NL = 2
EPS = 1e-5
S = 16384
P_FULL = S // MINP          # 4096 padded patch slots
SPC = S // N_CORES          # 2048 bytes/core for the entropy stage
TPC = 2048                  # tokens/core for the encoder stage
NPC = TPC // LMAX           # 128 patches/core
NSB = TPC // 128            # 16 superblocks (= token tiles)/core
BPS = 128 // LMAX           # 8 patches per superblock
CHUNK = N_CORES * NPC       # 1024 patches per encoder launch
NTT = TPC // 128            # 16 token tiles
NTF = TPC // 512            # 4 free tiles

dt = mybir.dt
F32 = dt.float32
BF16 = dt.bfloat16
AF = mybir.ActivationFunctionType
OP = mybir.AluOpType
AX = mybir.AxisListType
BF_NP = ml_dtypes.bfloat16

LN256 = float(np.log(np.float32(V)))


def _nc():
    return bacc.Bacc("TRN2", target_bir_lowering=False, debug=False,
                     num_devices=N_CORES)


# ======================================================================
# Entropy program: per core, 2048 bytes.
# ======================================================================
def build_ent(use_eb2, sim_proxy=False):
    nc = _nc()
    exT_d = nc.dram_tensor("exT", [D, SPC], BF16, kind="ExternalInput").ap()
    ew1T_d = nc.dram_tensor("ew1T", [D, 2 * D], BF16, kind="ExternalInput").ap()
    ew2T_d = nc.dram_tensor("ew2T", [2 * D, V], BF16, kind="ExternalInput").ap()
    eb1_d = nc.dram_tensor("eb1", [128, 8], F32, kind="ExternalInput").ap()
    eb2b_d = nc.dram_tensor("eb2b", [128, V], F32, kind="ExternalInput").ap()
    ent_d = nc.dram_tensor("ent", [SPC, 1], F32, kind="ExternalOutput").ap()

    NT = SPC // 128

    with tile.TileContext(nc) as tc:
        with tc.tile_pool(name="const", bufs=1) as const, \
             tc.tile_pool(name="hbuf", bufs=1) as hbuf:
            w1 = const.tile([128, 4, 2 * D], BF16, name="w1")
            for k in range(4):
                nc.sync.dma_start(w1[:, k, :], ew1T_d[ts(k, 128), :])
            w2 = const.tile([128, 8, V], BF16, name="w2")
            for k in range(8):
                nc.sync.dma_start(w2[:, k, :], ew2T_d[ts(k, 128), :])
            b1 = const.tile([128, 8], F32, name="b1")
            nc.sync.dma_start(b1[:], eb1_d[:])
            if sim_proxy:
                b1s = const.tile([128, 8], F32, name="b1s")
                nc.vector.tensor_scalar(b1s[:], b1[:], 1.702, None, op0=OP.mult)
            xk = const.tile([128, 4, SPC], BF16, name="xk")
            for k in range(4):
                nc.sync.dma_start(xk[:, k, :], exT_d[ts(k, 128), :])
            if use_eb2:
                b2 = const.tile([128, V], F32, name="b2")
                nc.sync.dma_start(b2[:], eb2b_d[:])

            hs = hbuf.tile([128, 8, SPC], BF16, name="hs")

            # h.T = gelu(W1 @ xb.T + b1) : [1024, 2048]
            with tc.tile_pool(name="psh", bufs=2, space="PSUM") as psh:
                for m in range(8):
                    ph = psh.tile([128, SPC], F32, name="ph")
                    for tf in range(SPC // 512):
                        for k in range(4):
                            nc.tensor.matmul(
                                ph[:, ts(tf, 512)],
                                lhsT=w1[:, k, ts(m, 128)],
                                rhs=xk[:, k, ts(tf, 512)],
                                start=(k == 0), stop=(k == 3))
                    if sim_proxy:
                        # CoreSim has no Gelu LUT: x*sigmoid(1.702x) proxy
                        sg = hbuf.tile([128, SPC], F32, name="sg", tag="sg")
                        nc.scalar.activation(sg[:], ph[:], AF.Sigmoid,
                                             bias=b1s[:, m:m + 1], scale=1.702)
                        uu = hbuf.tile([128, SPC], F32, name="uu", tag="uu")
                        nc.vector.tensor_scalar(uu[:], ph[:], b1[:, m:m + 1],
                                                None, op0=OP.add)
                        nc.vector.tensor_tensor(hs[:, m, :], uu[:], sg[:],
                                                OP.mult)
                    else:
                        nc.scalar.activation(hs[:, m, :], ph[:], AF.Gelu,
                                             bias=b1[:, m:m + 1], scale=1.0)

            # logits + entropy per 128-token tile
            with tc.tile_pool(name="psl", bufs=4, space="PSUM") as psl, \
                 tc.tile_pool(name="sc", bufs=4) as sc:
                for tt in range(NT):
                    pl = psl.tile([128, V], F32, name="pl")
                    for k in range(8):
                        nc.tensor.matmul(pl[:], lhsT=hs[:, k, ts(tt, 128)],
                                         rhs=w2[:, k, :],
                                         start=(k == 0), stop=(k == 7))
                    if use_eb2:
                        ladj = sc.tile([128, V], F32, name="ladj")
                        nc.vector.tensor_tensor(ladj[:], pl[:], b2[:], OP.add)
                        lsrc = ladj
                    else:
                        lsrc = pl
                    mx = sc.tile([128, 1], F32, name="mx")
                    nc.vector.tensor_reduce(mx[:], lsrc[:], axis=AX.X, op=OP.max)
                    tsb = sc.tile([128, V], F32, name="tsb")
                    nc.vector.tensor_scalar(tsb[:], lsrc[:], mx[:], None,
                                            op0=OP.subtract)
                    esb = sc.tile([128, V], F32, name="esb")
                    ssum = sc.tile([128, 1], F32, name="ssum")
                    nc.scalar.activation(esb[:], tsb[:], AF.Exp, accum_out=ssum[:])
                    scr = sc.tile([128, V], F32, name="scr")
                    nc.vector.tensor_tensor(scr[:], esb[:], tsb[:], OP.mult)
                    dot = sc.tile([128, 1], F32, name="dot")
                    nc.vector.tensor_reduce(dot[:], scr[:], axis=AX.X, op=OP.add)
                    rsum = sc.tile([128, 1], F32, name="rsum")
                    nc.vector.reciprocal(rsum[:], ssum[:])
                    lsum = sc.tile([128, 1], F32, name="lsum")
                    nc.scalar.activation(lsum[:], ssum[:], AF.Ln)
                    t1 = sc.tile([128, 1], F32, name="t1")
                    nc.vector.tensor_tensor(t1[:], dot[:], rsum[:], OP.mult)
                    t2 = sc.tile([128, 1], F32, name="t2")
                    nc.vector.tensor_tensor(t2[:], lsum[:], t1[:], OP.subtract)
                    ent_t = sc.tile([128, 1], F32, name="ent_t")
                    nc.vector.tensor_scalar(ent_t[:], t2[:], 1.0 / LN256, None,
                                            op0=OP.mult)
                    nc.sync.dma_start(ent_d[ts(tt, 128), :], ent_t[:])

    nc.compile()
    return nc


# ======================================================================
# Encoder program: per core, 128 patches = 2048 tokens, 2 layers.
# ======================================================================
def build_enc(use_bo, use_b2, use_aff1, use_aff2):
    nc = _nc()
    g = {}
    g["xT_d"] = nc.dram_tensor("xT", [D, TPC], BF16, kind="ExternalInput").ap()
    g["xN_d"] = nc.dram_tensor("xN", [TPC, D], F32, kind="ExternalInput").ap()
    g["wq_d"] = nc.dram_tensor("wq", [NL, D, D], BF16, kind="ExternalInput").ap()
    g["wk_d"] = nc.dram_tensor("wk", [NL, D, D], BF16, kind="ExternalInput").ap()
    g["wv_d"] = nc.dram_tensor("wv", [NL, D, D], BF16, kind="ExternalInput").ap()
    g["wo_d"] = nc.dram_tensor("wo", [NL, D, D], BF16, kind="ExternalInput").ap()
    g["w1_d"] = nc.dram_tensor("w1", [NL, D, F], BF16, kind="ExternalInput").ap()
    g["w2_d"] = nc.dram_tensor("w2", [NL, F, D], BF16, kind="ExternalInput").ap()
    g["bqk_d"] = nc.dram_tensor("bqk", [128, NL, 2, 4], F32, kind="ExternalInput").ap()
    g["bvb_d"] = nc.dram_tensor("bvb", [128, NL, D], F32, kind="ExternalInput").ap()
    g["b1_d"] = nc.dram_tensor("b1f", [128, NL, 16], F32, kind="ExternalInput").ap()
    g["bob_d"] = nc.dram_tensor("bob", [128, NL, D], F32, kind="ExternalInput").ap()
    g["b2b_d"] = nc.dram_tensor("b2b", [128, NL, D], F32, kind="ExternalInput").ap()
    g["g1b_d"] = nc.dram_tensor("g1b", [128, NL, D], F32, kind="ExternalInput").ap()
    g["h1b_d"] = nc.dram_tensor("h1b", [128, NL, D], F32, kind="ExternalInput").ap()
    g["g2b_d"] = nc.dram_tensor("g2b", [128, D], F32, kind="ExternalInput").ap()
    g["h2b_d"] = nc.dram_tensor("h2b", [128, D], F32, kind="ExternalInput").ap()
    g["kvbd_d"] = nc.dram_tensor("kvbd", [NSB, 128, 128], BF16, kind="ExternalInput").ap()
    g["poolm_d"] = nc.dram_tensor("poolm", [NSB, 128, 128], F32, kind="ExternalInput").ap()
    g["emb_d"] = nc.dram_tensor("emb", [NPC, D], F32, kind="ExternalOutput").ap()

    with tile.TileContext(nc) as tc:
        _enc_body(nc, tc, g, use_bo, use_b2, use_aff1, use_aff2)
    nc.compile()
    return nc


def _enc_body(nc, tc, g, use_bo, use_b2, use_aff1, use_aff2):
    const = tc.alloc_tile_pool(name="const", bufs=1)
    bqk = const.tile([128, NL, 2, 4], F32, name="bqk")
    nc.sync.dma_start(bqk[:], g["bqk_d"][:])
    bvb = const.tile([128, NL, D], F32, name="bvb")
    nc.sync.dma_start(bvb[:], g["bvb_d"][:])
    b1f = const.tile([128, NL, 16], F32, name="b1f")
    nc.sync.dma_start(b1f[:], g["b1_d"][:])
    bob = h2b = b2b = g1b = h1b = g2b = None
    if use_bo:
        bob = const.tile([128, NL, D], F32, name="bob")
        nc.sync.dma_start(bob[:], g["bob_d"][:])
    if use_b2:
        b2b = const.tile([128, NL, D], F32, name="b2b")
        nc.sync.dma_start(b2b[:], g["b2b_d"][:])
    if use_aff1:
        g1b = const.tile([128, NL, D], F32, name="g1b")
        nc.sync.dma_start(g1b[:], g["g1b_d"][:])
        h1b = const.tile([128, NL, D], F32, name="h1b")
        nc.sync.dma_start(h1b[:], g["h1b_d"][:])
    if use_aff2:
        g2b = const.tile([128, D], F32, name="g2b")
        nc.sync.dma_start(g2b[:], g["g2b_d"][:])
        h2b = const.tile([128, D], F32, name="h2b")
        nc.sync.dma_start(h2b[:], g["h2b_d"][:])
    kvbd = const.tile([128, NSB, 128], BF16, name="kvbd")
    nc.sync.dma_start(kvbd[:], g["kvbd_d"].rearrange("s l m -> l s m"))
    ident = const.tile([128, 128], BF16, name="ident")
    make_identity(nc, ident[:])
    epsb = const.tile([128, 1], F32, name="epsb")
    nc.gpsimd.memset(epsb[:], EPS)

    # inter-layer activations round-trip through DRAM
    dram = tc.alloc_tile_pool(name="dram", bufs=1, space="DRAM")
    x2T_dram = dram.tile([4, 128, TPC], BF16, name="x2T_dram")
    x2N_dram = dram.tile([NTT, 128, D], BF16, name="x2N_dram")

    wpool = tc.alloc_tile_pool(name="wts", bufs=1)
    poolm_pool = tc.alloc_tile_pool(name="poolmp", bufs=2)
    xts_pool = tc.alloc_tile_pool(name="xts", bufs=2)
    xn_pool = tc.alloc_tile_pool(name="xn", bufs=4)
    qkv_pool = tc.alloc_tile_pool(name="qkv", bufs=2)
    att_pool = tc.alloc_tile_pool(name="att", bufs=2)
    ln_pool = tc.alloc_tile_pool(name="ln", bufs=3)
    x1f_pool = tc.alloc_tile_pool(name="x1f", bufs=5)
    x1t_pool = tc.alloc_tile_pool(name="x1t", bufs=2)
    h1_pool = tc.alloc_tile_pool(name="h1", bufs=1)
    stage_pool = tc.alloc_tile_pool(name="stage", bufs=4)
    out_pool = tc.alloc_tile_pool(name="outp", bufs=1)

    ps512 = tc.alloc_tile_pool(name="ps512", bufs=2, space="PSUM")
    ps_s = tc.alloc_tile_pool(name="ps_s", bufs=1, space="PSUM")
    ps_t = tc.alloc_tile_pool(name="ps_t", bufs=2, space="PSUM")
    ps_o = tc.alloc_tile_pool(name="ps_o", bufs=1, space="PSUM")
    ps_pool = tc.alloc_tile_pool(name="ps_pool", bufs=1, space="PSUM")

    emb_ps = ps_pool.tile([128, D], F32, name="emb_ps")

    def layer(l):
        # per-layer weights (slots shared between layers via tags)
        wq = wpool.tile([128, 4, D], BF16, name="wq", tag="wq")
        wk = wpool.tile([128, 4, D], BF16, name="wk", tag="wk")
        wv = wpool.tile([128, 4, D], BF16, name="wv", tag="wv")
        wo = wpool.tile([128, 4, D], BF16, name="wo", tag="wo")
        for k in range(4):
            nc.sync.dma_start(wq[:, k, :], g["wq_d"][l, ts(k, 128), :])
            nc.sync.dma_start(wk[:, k, :], g["wk_d"][l, ts(k, 128), :])
            nc.sync.dma_start(wv[:, k, :], g["wv_d"][l, ts(k, 128), :])
            nc.sync.dma_start(wo[:, k, :], g["wo_d"][l, ts(k, 128), :])
        w1 = wpool.tile([128, 4, F], BF16, name="w1", tag="w1")
        for k in range(4):
            nc.sync.dma_start(w1[:, k, :], g["w1_d"][l, ts(k, 128), :])
        w2 = wpool.tile([128, 16, D], BF16, name="w2", tag="w2")
        for k in range(16):
            nc.sync.dma_start(w2[:, k, :], g["w2_d"][l, ts(k, 128), :])

        for tf in range(NTF):
            # ---- load transposed input for these 512 tokens ----
            xts = xts_pool.tile([128, 4, 512], BF16, name="xts")
            for k in range(4):
                if l == 0:
                    nc.sync.dma_start(xts[:, k, :],
                                      g["xT_d"][ts(k, 128), ts(tf, 512)])
                else:
                    nc.sync.dma_start(xts[:, k, :],
                                      x2T_dram[k, :, ts(tf, 512)])
            # ---- QKV projections for 512 tokens ----
            qt = qkv_pool.tile([128, 4, 512], BF16, name="qt")
            kt = qkv_pool.tile([128, 4, 512], BF16, name="kt")
            vn = qkv_pool.tile([128, 4, D], BF16, name="vn")
            for m in range(4):
                pq = ps512.tile([128, 512], F32, name="pq", tag="ps512")
                for k in range(4):
                    nc.tensor.matmul(pq[:], lhsT=wq[:, k, ts(m, 128)],
                                     rhs=xts[:, k, :],
                                     start=(k == 0), stop=(k == 3))
                nc.vector.tensor_scalar(qt[:, m, :], pq[:], bqk[:, l, 0, m:m + 1],
                                        None, op0=OP.add)
            for m in range(4):
                pk = ps512.tile([128, 512], F32, name="pk", tag="ps512")
                for k in range(4):
                    nc.tensor.matmul(pk[:], lhsT=wk[:, k, ts(m, 128)],
                                     rhs=xts[:, k, :],
                                     start=(k == 0), stop=(k == 3))
                nc.vector.tensor_scalar(kt[:, m, :], pk[:], bqk[:, l, 1, m:m + 1],
                                        None, op0=OP.add)
            for i in range(4):
                pv = ps512.tile([128, 512], F32, name="pv", tag="ps512")
                for k in range(4):
                    nc.tensor.matmul(pv[:], lhsT=xts[:, k, ts(i, 128)],
                                     rhs=wv[:, k, :],
                                     start=(k == 0), stop=(k == 3))
                nc.vector.tensor_tensor(vn[:, i, :], pv[:], bvb[:, l, :], OP.add)

            # odd heads' Q/K rows moved to partition base 0 (matmul inputs
            # with base_partition=64 are broken on HW)
            qo = qkv_pool.tile([64, 4, 512], BF16, name="qo")
            ko = qkv_pool.tile([64, 4, 512], BF16, name="ko")
            for m in range(4):
                nc.sync.dma_start(qo[:, m, :], qt[ds(64, 64), m, :])
                nc.sync.dma_start(ko[:, m, :], kt[ds(64, 64), m, :])

            x1t = x1t_pool.tile([128, 4, 512], BF16, name="x1t")
            x1f_tiles = []
            # ---- attention per superblock (128 tokens = 8 patches) ----
            for i in range(4):
                tt = tf * 4 + i
                pss = ps_s.tile([128, 8, 128], F32, name="pss")
                for h in range(H):
                    qsrc = qt if h % 2 == 0 else qo
                    ksrc = kt if h % 2 == 0 else ko
                    nc.tensor.matmul(
                        pss[:, h, :],
                        lhsT=qsrc[ds(0, 64), h // 2, ts(i, 128)],
                        rhs=ksrc[ds(0, 64), h // 2, ts(i, 128)],
                        start=True, stop=True)
                psb = att_pool.tile([128, 8, 128], BF16, name="psb")
                nc.scalar.activation(psb[:], pss[:], AF.Exp, scale=0.125)
                pm = att_pool.tile([128, 8, 128], BF16, name="pm")
                nc.vector.tensor_tensor(
                    pm[:], psb[:],
                    kvbd[:, tt, :].unsqueeze(1).broadcast_to([128, 8, 128]),
                    OP.mult)
                rs = att_pool.tile([128, 8], F32, name="rs")
                nc.vector.tensor_reduce(rs[:], pm[:], axis=AX.X, op=OP.add)
                rr = att_pool.tile([128, 8], F32, name="rr")
                nc.vector.reciprocal(rr[:], rs[:])
                pn = att_pool.tile([128, 8, 128], BF16, name="pn")
                nc.vector.tensor_tensor(
                    pn[:], pm[:], rr.unsqueeze(2).broadcast_to([128, 8, 128]),
                    OP.mult)
                pnT = att_pool.tile([128, 8, 128], BF16, name="pnT")
                for h in range(H):
                    pst = ps_t.tile([128, 128], BF16, name="pst", tag="pst")
                    nc.tensor.transpose(pst[:], pn[:, h, :], ident[:])
                    nc.vector.tensor_copy(pnT[:, h, :], pst[:])
                pso = ps_o.tile([128, 512], F32, name="pso")
                for h in range(H):
                    nc.tensor.matmul(
                        pso[ds(64 * (h % 2), 64), ts(h // 2, 128)],
                        lhsT=vn[:, i, ds(64 * h, 64)],
                        rhs=pnT[:, h, :], start=True, stop=True)
                ot = att_pool.tile([128, 4, 128], BF16, name="ot")
                nc.vector.tensor_copy(ot.rearrange("p a b -> p (a b)"), pso[:])
                # ---- output projection + residual + LN1 ----
                pao = ps512.tile([128, 512], F32, name="pao", tag="ps512")
                for kb in range(4):
                    nc.tensor.matmul(pao[:], lhsT=ot[:, kb, :],
                                     rhs=wo[:, kb, :],
                                     start=(kb == 0), stop=(kb == 3))
                if l == 0:
                    res = xn_pool.tile([128, D], F32, name="xn", tag="xn")
                    nc.sync.dma_start(res[:], g["xN_d"][ts(tt, 128), :])
                else:
                    res = xn_pool.tile([128, D], BF16, name="xn2", tag="xn2")
                    nc.sync.dma_start(res[:], x2N_dram[tt, :, :])
                r1 = ln_pool.tile([128, D], F32, name="r1")
                nc.vector.scalar_tensor_tensor(
                    r1[:], in0=pao[:], scalar=1.0, in1=res[:],
                    op0=OP.mult, op1=OP.add)
                if use_bo:
                    nc.vector.tensor_tensor(r1[:], r1[:], bob[:, l, :], OP.add)
                aff1 = (g1b[:, l, :], h1b[:, l, :]) if use_aff1 else None
                x1b, x1f = _ln(nc, ln_pool, x1f_pool, r1, aff1, want_f32=True, epsb=epsb)
                x1f_tiles.append(x1f)
                for j in range(4):
                    pxt = ps_t.tile([128, 128], BF16, name="pxt", tag="pst")
                    nc.tensor.transpose(pxt[:], x1b[:, ts(j, 128)], ident[:])
                    nc.scalar.copy(x1t[:, j, ts(i, 128)], pxt[:])

            # ---- FFN for 512 tokens ----
            h1 = h1_pool.tile([128, 16, 512], BF16, name="h1")
            for m in range(16):
                pf = ps512.tile([128, 512], F32, name="pf", tag="ps512")
                for k in range(4):
                    nc.tensor.matmul(pf[:], lhsT=w1[:, k, ts(m, 128)],
                                     rhs=x1t[:, k, :],
                                     start=(k == 0), stop=(k == 3))
                nc.scalar.activation(h1[:, m, :], pf[:], AF.Relu,
                                     bias=b1f[:, l, m:m + 1])
            for i in range(4):
                tt = tf * 4 + i
                pf2 = ps512.tile([128, 512], F32, name="pf2", tag="ps512")
                for k in range(16):
                    nc.tensor.matmul(pf2[:], lhsT=h1[:, k, ts(i, 128)],
                                     rhs=w2[:, k, :],
                                     start=(k == 0), stop=(k == 15))
                r2 = ln_pool.tile([128, D], F32, name="r2")
                nc.vector.scalar_tensor_tensor(
                    r2[:], in0=pf2[:], scalar=1.0, in1=x1f_tiles[i][:],
                    op0=OP.mult, op1=OP.add)
                if use_b2:
                    nc.vector.tensor_tensor(r2[:], r2[:], b2b[:, l, :], OP.add)
                if l == 0:
                    # inner LN2: bf16 out -> stream to DRAM for layer 2
                    aff2 = (g2b[:], h2b[:]) if use_aff2 else None
                    x2b, _ = _ln(nc, ln_pool, x1f_pool, r2, aff2, want_f32=False, epsb=epsb)
                    nc.sync.dma_start(x2N_dram[tt, :, :], x2b)
                    for j in range(4):
                        pxt2 = ps_t.tile([128, 128], BF16, name="pxt2", tag="pst")
                        nc.tensor.transpose(pxt2[:], x2b[:, ts(j, 128)], ident[:])
                        st = stage_pool.tile([128, 128], BF16, name="st")
                        nc.vector.tensor_copy(st[:], pxt2[:])
                        nc.sync.dma_start(x2T_dram[j, :, ts(tt, 128)], st[:])
                else:
                    # final LN (affine applied on host) + pooling
                    x2f = ln_pool.tile([128, D], F32, name="x2f")
                    _ln(nc, ln_pool, x1f_pool, r2, None, want_f32=False,
                        epsb=epsb, out_norm=x2f[:])
                    pw = poolm_pool.tile([128, 128], F32, name="pw")
                    nc.sync.dma_start(pw[:], g["poolm_d"][tt, :, :])
                    nc.tensor.matmul(emb_ps[:], lhsT=pw[:], rhs=x2f[:],
                                     start=(tt == 0), stop=(tt == NTT - 1))

    layer(0)
    layer(1)

    emb_sb = out_pool.tile([128, D], F32, name="emb_sb")
    nc.vector.tensor_copy(emb_sb[:], emb_ps[:])
    nc.sync.dma_start(g["emb_d"][:], emb_sb[:])

    for p in (ps_pool, ps_o, ps_t, ps_s, ps512, out_pool, stage_pool, h1_pool,
              x1t_pool, x1f_pool, ln_pool, att_pool, qkv_pool, xn_pool,
              xts_pool, poolm_pool, wpool, dram, const):
        p.release()


def _ln(nc, ln_pool, f_pool, r, aff, want_f32, epsb, out_norm=None):
    """LayerNorm over the free dim of r [128, D] f32.

    Returns (bf16_ap, f32_ap_or_None). rstd computed as exp(-0.5*log(var+eps))
    to stay inside the natural_log_exp ACT table set (no sqrt)."""
    st6 = ln_pool.tile([128, 6], F32, name="st6")
    nc.vector.bn_stats(st6[:], r[:])
    mv = ln_pool.tile([128, 2], F32, name="mv")
    nc.vector.bn_aggr(mv[:], st6[:])
    lv = ln_pool.tile([128, 1], F32, name="lv")
    nc.scalar.activation(lv[:], mv[:, 1:2], AF.Ln, bias=epsb[:])
    lh = ln_pool.tile([128, 1], F32, name="lh")
    nc.vector.tensor_scalar(lh[:], lv[:], -0.5, None, op0=OP.mult)
    rstd = ln_pool.tile([128, 1], F32, name="rstd")
    nc.scalar.activation(rstd[:], lh[:], AF.Exp)
    if out_norm is None:
        xb = ln_pool.tile([128, D], BF16, name="xb")
        out_norm = xb[:]
    if aff is None:
        nc.vector.tensor_scalar(out_norm, r[:], mv[:, 0:1], rstd[:],
                                op0=OP.subtract, op1=OP.mult)
    else:
        gg, bb = aff
        pre = ln_pool.tile([128, D], F32, name="pre")
        nc.vector.tensor_scalar(pre[:], r[:], mv[:, 0:1], rstd[:],
                                op0=OP.subtract, op1=OP.mult)
        mid = ln_pool.tile([128, D], F32, name="mid")
        nc.vector.tensor_tensor(mid[:], pre[:], gg, OP.mult)
        nc.vector.tensor_tensor(out_norm, mid[:], bb, OP.add)
    xf = None
    if want_f32:
        xf = f_pool.tile([128, D], F32, name="xf")
        if aff is None:
            nc.vector.tensor_scalar(xf[:], r[:], mv[:, 0:1], rstd[:],
                                    op0=OP.subtract, op1=OP.mult)
        else:
            nc.vector.tensor_tensor(xf[:], mid[:], aff[1], OP.add)
    return out_norm, xf


# ======================================================================
# Host-side orchestration
# ======================================================================
_PROGS = {}


def _get_prog(kind, flags):
    key = (kind, flags)
    if key not in _PROGS:
        _PROGS[key] = build_ent(*flags) if kind == "ent" else build_enc(*flags)
    return _PROGS[key]


def _segment(ent):
    """Entropy-gated segmentation. Equivalent to the reference scan."""
    low = ent < THR
    idx = np.arange(S)
    nlow = np.where(low, idx, S)
    nlow = np.minimum.accumulate(nlow[::-1])[::-1]  # first low index >= i
    starts, lens = [], []
    s = 0
    while s < S:
        j = s + MINP - 1
        b = int(nlow[j]) if j < S else S
        b = min(b, s + LMAX - 1, S - 1)
        starts.append(s)
        lens.append(b - s + 1)
        s = b + 1
    return np.array(starts), np.array(lens)


# set PROFILE=True (e.g. from test.py) to capture NTFF exec times per launch
PROFILE = False
LAST_EXEC_NS = []


def _run_spmd(nc, in_maps, sim=False):
    if sim:
        from concourse.bass_interp import CoreSim
        outs = []
        for m in in_maps:
            cs = CoreSim(nc)
            for k, v in m.items():
                cs.tensor(k)[:] = v
            cs.simulate(check_with_hw=False)
            names = [a.memorylocations[0].name
                     for a in nc.m.functions[0].allocations
                     if isinstance(a, mybir.MemoryLocationSet)
                     and a.kind == "ExternalOutput"]
            outs.append({n: np.array(cs.tensor(n)) for n in names})
        return outs
    res = bass_utils.run_bass_kernel_spmd(nc, in_maps, list(range(N_CORES)),
                                          trace=PROFILE)
    if PROFILE:
        LAST_EXEC_NS.append(res.exec_time_ns)
    return res.results


def kernel(byte_sequence, byte_emb, ent_w1, ent_b1, ent_w2, ent_b2,
           attn_wqkv, attn_bqkv, attn_wo, attn_bo, ln1_g, ln1_b,
           ffn_w1, ffn_b1, ffn_w2, ffn_b2, ln2_g, ln2_b, _sim=False):
    seq = np.asarray(byte_sequence)
    emb_t = np.ascontiguousarray(np.asarray(byte_emb, dtype=np.float32))
    f32 = lambda a: np.asarray(a, dtype=np.float32)
    ent_w1, ent_b1, ent_w2, ent_b2 = map(f32, (ent_w1, ent_b1, ent_w2, ent_b2))
    attn_wqkv, attn_bqkv, attn_wo, attn_bo = map(
        f32, (attn_wqkv, attn_bqkv, attn_wo, attn_bo))
    ln1_g, ln1_b, ln2_g, ln2_b = map(f32, (ln1_g, ln1_b, ln2_g, ln2_b))
    ffn_w1, ffn_b1, ffn_w2, ffn_b2 = map(f32, (ffn_w1, ffn_b1, ffn_w2, ffn_b2))

    # ---------------- stage 1: entropy on device ----------------
    use_eb2 = bool(np.any(ent_b2 != 0))
    nc_ent = _get_prog("ent", (use_eb2,))
    xbT = np.ascontiguousarray(emb_t[seq].T).astype(BF_NP)   # [512, 16384]
    ent_in = {
        "ew1T": np.ascontiguousarray(ent_w1.T).astype(BF_NP),
        "ew2T": np.ascontiguousarray(ent_w2.T).astype(BF_NP),
        "eb1": np.ascontiguousarray(ent_b1.reshape(8, 128).T),
        "eb2b": np.ascontiguousarray(np.broadcast_to(ent_b2, (128, V))),
    }
    in_maps = [dict(ent_in, exT=np.ascontiguousarray(xbT[:, c * SPC:(c + 1) * SPC]))
               for c in range(N_CORES)]
    outs = _run_spmd(nc_ent, in_maps, sim=_sim)
    ent = np.concatenate([o["ent"][:, 0] for o in outs])      # [16384]

    # ---------------- stage 2: segmentation on host ----------------
    starts, lens = _segment(ent)
    n_real = len(starts)
    lengths = np.zeros(P_FULL, np.int32)
    lengths[:n_real] = lens
    tokens = np.zeros((P_FULL, LMAX), np.int64)
    for p in range(n_real):
        tokens[p, :lens[p]] = seq[starts[p]:starts[p] + lens[p]]

    # ---------------- stage 3: encoder on device ----------------
    use_bo = bool(np.any(attn_bo != 0))
    use_b2 = bool(np.any(ffn_b2 != 0))
    use_aff1 = bool(np.any(ln1_g != 1) or np.any(ln1_b != 0))
    use_aff2 = bool(np.any(ln2_g[0] != 1) or np.any(ln2_b[0] != 0))
    nc_enc = _get_prog("enc", (use_bo, use_b2, use_aff1, use_aff2))

    tr = lambda a: np.ascontiguousarray(a.transpose(0, 2, 1)).astype(BF_NP)
    bc = lambda a: np.ascontiguousarray(
        np.broadcast_to(a[:, None, :], (NL, 128, a.shape[1])).transpose(1, 0, 2))
    wq_h, wk_h, wv_h = (tr(attn_wqkv[:, i * D:(i + 1) * D, :]) for i in range(3))
    common = {
        "wq": wq_h, "wk": wk_h, "wv": wv_h, "wo": tr(attn_wo),
        "w1": tr(ffn_w1), "w2": tr(ffn_w2),
        "bqk": np.ascontiguousarray(
            np.stack([attn_bqkv[:, :D].reshape(NL, 4, 128),
                      attn_bqkv[:, D:2 * D].reshape(NL, 4, 128)],
                     axis=1).transpose(3, 0, 1, 2)),
        "bvb": bc(attn_bqkv[:, 2 * D:]),
        "b1f": np.ascontiguousarray(ffn_b1.reshape(NL, 16, 128).transpose(2, 0, 1)),
        "bob": bc(attn_bo), "b2b": bc(ffn_b2),
        "g1b": bc(ln1_g), "h1b": bc(ln1_b),
        "g2b": np.ascontiguousarray(np.broadcast_to(ln2_g[0], (128, D))),
        "h2b": np.ascontiguousarray(np.broadcast_to(ln2_b[0], (128, D))),
    }

    blk = (np.arange(128)[:, None] // LMAX) == (np.arange(128)[None, :] // LMAX)
    emb_out = np.zeros((P_FULL, D), np.float32)
    n_chunks = max(1, -(-n_real // CHUNK))
    for ci in range(n_chunks):
        in_maps = []
        for c in range(N_CORES):
            p0 = ci * CHUNK + c * NPC
            ptok = tokens[p0:p0 + NPC]
            plen = lengths[p0:p0 + NPC].astype(np.int64)
            xg = emb_t[ptok.reshape(-1)]                      # [2048, 512] f32
            # key mask: real patches mask invalid keys; empty slots use the
            # full block (finite garbage, excluded by pooling)
            mlen = np.where(plen == 0, LMAX, plen)
            validf = (np.arange(LMAX)[None, :] < mlen[:, None]).reshape(NSB, 128)
            kv = (blk[None, :, :] & validf[:, None, :]).astype(BF_NP)
            poolw = np.zeros((NSB, 128, 128), np.float32)
            pvalid = (np.arange(LMAX)[None, :] < plen[:, None]) / \
                np.maximum(plen, 1)[:, None]
            for pp in range(NPC):
                sbi, t0 = pp // BPS, (pp % BPS) * LMAX
                poolw[sbi, t0:t0 + LMAX, pp] = pvalid[pp]
            in_maps.append(dict(
                common,
                xT=np.ascontiguousarray(xg.T).astype(BF_NP),
                xN=xg, kvbd=kv, poolm=poolw))
        outs = _run_spmd(nc_enc, in_maps, sim=_sim)
        for c in range(N_CORES):
            p0 = ci * CHUNK + c * NPC
            hi = min(p0 + NPC, P_FULL)
            if p0 < P_FULL:
                emb_out[p0:hi] = outs[c]["emb"][:hi - p0]

    # final-layer LN affine on host (pooling is linear)
    if np.any(ln2_g[1] != 1) or np.any(ln2_b[1] != 0):
        emb_out[:n_real] = emb_out[:n_real] * ln2_g[1] + ln2_b[1]
    emb_out[n_real:] = 0.0
    return emb_out, lengths
